# revision 7
# baseline (speedup 1.0000x reference)
"""DTW layer (short kernel) Trainium2 Bass kernel.

Problem: x (B=8, C=8, L=4096) f32, kernels (F=32, K=10) f32.
For each (b, c, f, w): DTW cost between kernels[f] (len 10) and window
x[b, c, 5w : 5w+20], for w in [0, 815). Output (B, C*F, 815) f32.

Sharding: data-parallel over batch - core b computes batch b entirely
(C*F = 256 (c,f) combos = 2 partition chunks of 128).

Algorithm (per core): the DTW row recurrence
    row_i[j] = D[i,j] + min(row_i[j-1], row_{i-1}[j], row_{i-1}[j-1])
is computed for 128 (c,f) combos at once (partition dim) and a chunk of
windows laid out along the free dim as [w, 21] segments (1 separator +
20 cells).  Per row:
  - ACT computes local costs D[w, 1+j] = (x[5w+j] - k_i)^2 via
    activation(Square, bias=-k_i) with an overlapping strided input AP.
  - DVE computes m[t] = min(S_prev[t], S_prev[t-1]) (3-way-min helper)
    and then one tensor_tensor_scan per row:
    state = min(m[t], state) + D[t], with a BIG value in the separator
    column of D forcing a carry reset between windows.

Perf notes (hardware-measured in situ):
  - All DP tensors (S, D) are fp16: the shifted-min tensor_tensor runs
    ~1.5x faster (2x_1p packing) and scan speed is dtype-neutral.
    Max DP value ~70 << fp16 max; rel err ~1e-3 << 2e-2 gate.
  - The m helper writes INTO the scan's own output buffer S[i%2] (row
    i-2's values are dead by then) and the scan runs in place
    (data0 == out): one less distinct SBUF stream per scan (~6%).
  - Pair-interleaved units with same-op-type grouping (m,m,scan,scan)
    to hide cross-engine latency and reduce op-type switches.
  - m is computed on cell positions only (3D windowed views), skipping
    separator columns.

Raw bass (no Tile framework): engines are programmed directly with
standalone wait_ge instructions and per-engine semaphores.
"""

from contextlib import ExitStack

import numpy as np

import concourse.bass as bass
import concourse.mybir as mybir
from concourse.bass_utils import run_bass_kernel_spmd

# Problem constants (hardcoded per harness contract)
B, C, L = 8, 8, 4096
F, K = 32, 10
PROC, STEP = 20, 5
NW = 815          # windows actually computed == chan_outlen
SEG = PROC + 1    # 1 separator + 20 cells
NWC = 136         # windows per chunk; 6 chunks = 816 >= 815
NCHUNK = 6
TFREE = NWC * SEG # 2856 scan length
BIG = 30000.0     # fp16-safe sentinel (max DP value ~70)
SLOTS = 2
UNITS = [(cc, wc) for cc in range(2) for wc in range(NCHUNK)]

F32 = mybir.dt.float32
F16 = mybir.dt.float16


def _build_nc(reps: int = 1) -> bass.Bass:
    # detect_race_conditions=False: CoreSim's detector does not model
    # same-engine program order, which this kernel relies on throughout.
    nc = bass.Bass("TRN2", debug=False, detect_race_conditions=False)
    x_d = nc.dram_tensor("x", [C, L], F32, kind="ExternalInput").ap()
    k_d = nc.dram_tensor("negk", [F, K], F32, kind="ExternalInput").ap()
    out_d = nc.dram_tensor("out", [C * F, NWC * NCHUNK], F32,
                           kind="ExternalOutput").ap()

    UNITS_R = UNITS * reps
    big = BIG

    # --- semaphore bookkeeping (python-side op counts) ---
    # DVE emission order: init memsets, then per unit pair, per row i:
    # m(A,i), m(B,i), scan(A,i), scan(B,i)  (m omitted for i=0).
    dve_ops = []  # ("m"|"scan", u, i)
    nu = len(UNITS) * reps
    for base in range(0, nu, 2):
        pair = [base] + ([base + 1] if base + 1 < nu else [])
        for i in range(K):
            if i > 0:
                for u in pair:
                    dve_ops.append(("m", u, i))
            for u in pair:
                dve_ops.append(("scan", u, i))
    N_INIT_MS = 8 + 2 * SLOTS  # memsets before the op stream
    _scan_pos = {(u, i): N_INIT_MS + n + 1
                 for n, (kind, u, i) in enumerate(dve_ops)
                 if kind == "scan"}

    def dve_through_scan(u, i):
        return _scan_pos[(u, i)]

    # ACT order: pair-interleaved to match the DVE order: per pair,
    # squares (u0,i),(u1,i) for each i, then both extract copies.
    act_ops = []  # ("sq"|"cp", u, i)
    for base in range(0, nu, 2):
        pair = [base] + ([base + 1] if base + 1 < nu else [])
        for i in range(K):
            for u in pair:
                act_ops.append(("sq", u, i))
        for u in pair:
            act_ops.append(("cp", u, 0))
    _sq_pos = {(u, i): n + 1 for n, (kind, u, i) in enumerate(act_ops)
               if kind == "sq"}
    _cp_pos = {u: n + 1 for n, (kind, u, i) in enumerate(act_ops)
               if kind == "cp"}

    def act_through_square(u, i):
        return _sq_pos[(u, i)]

    def act_through_copy(u):
        return _cp_pos[u]

    def dma_through_out(u):  # X1 init DMA then one out-DMA per unit
        return 16 * (2 + u)

    with ExitStack() as ctx:
        sb = lambda shape, name, dt: ctx.enter_context(
            nc.sbuf_tensor(name, shape, dt))
        X = [sb([128, L], f"Xt{cc}", F32) for cc in range(2)]
        negK = sb([128, K], "negKt", F32)
        m0 = sb([128, TFREE], "m0t", F16)
        S = [[sb([128, TFREE], f"St{s}_{i}", F16) for i in range(2)]
             for s in range(SLOTS)]
        D = [[sb([128, TFREE], f"Dt{s}_{i}", F16) for i in range(2)]
             for s in range(SLOTS)]
        OB = [sb([128, NWC], f"OBt{s}", F32) for s in range(SLOTS)]

        dma_sem = ctx.enter_context(nc.semaphore("dma_sem"))
        dma0_sem = ctx.enter_context(nc.semaphore("dma0_sem"))
        act_sem = ctx.enter_context(nc.semaphore("act_sem"))
        dve_sem = ctx.enter_context(nc.semaphore("dve_sem"))
        block = ctx.enter_context(nc.Block())

        @block.sync
        def _(sync):
            # negK + X0 first so cc0 compute starts before X1 lands.
            # X[cc] partition p holds x[4*cc + p//32, :] (source AP
            # replicates each channel row 32x via a step-0 dim)
            ksrc = bass.AP(k_d.tensor, 0, [[0, 4], [K, F], [1, K]])
            sync.dma_start(negK.ap(), ksrc).then_inc(dma0_sem, 16)
            for cc in range(2):
                src = bass.AP(x_d.tensor, 4 * cc * L,
                              [[L, 4], [0, 32], [1, L]])
                sync.dma_start(X[cc].ap(), src).then_inc(
                    dma0_sem if cc == 0 else dma_sem, 16)
            for u, (cc, wc) in enumerate(UNITS_R):
                s = u % SLOTS
                sync.wait_ge(act_sem, act_through_copy(u))
                sync.dma_start(
                    out_d[128 * cc:128 * (cc + 1),
                          NWC * wc:NWC * (wc + 1)],
                    OB[s].ap()).then_inc(dma_sem, 16)

        @block.vector
        def _(vector):
            # init: m0 = BIG with 0 at each segment's cell j=0 (offset 1);
            # D separator columns BIG; S separator columns BIG (the
            # in-place scan reads them as data0 on the first unit).
            vector.memset(m0.ap(), big).then_inc(dve_sem, 1)
            m0_seg = m0.ap().rearrange("p (w s) -> p w s", s=SEG)
            vector.memset(m0_seg[:, :, 1], 0.0).then_inc(dve_sem, 1)
            for s in range(SLOTS):
                for i in range(2):
                    d_seg = D[s][i].ap().rearrange("p (w s) -> p w s", s=SEG)
                    vector.memset(d_seg[:, :, 0], big).then_inc(dve_sem, 1)
                    s_seg = S[s][i].ap().rearrange("p (w s) -> p w s", s=SEG)
                    vector.memset(s_seg[:, :, 0], big).then_inc(dve_sem, 1)
            # pad memset count to N_INIT_MS
            for _ in range(N_INIT_MS - 2 - 4 * SLOTS):
                vector.memset(m0_seg[:, :1, 1], 0.0).then_inc(dve_sem, 1)
            act_waited = 0
            for kind, u, i in dve_ops:
                s = u % SLOTS
                if kind == "m":
                    # m into the scan's own output buffer (cells only);
                    # row i-2's values there are dead.  Guard the one
                    # buffer ACT extract-reads (S[s][1]) against the
                    # previous unit's pending cp.
                    if i == 1 and u >= SLOTS:
                        need = act_through_copy(u - SLOTS)
                        if need > act_waited:
                            vector.wait_ge(act_sem, need)
                            act_waited = need
                    prev = S[s][(i - 1) % 2].ap()
                    dst = S[s][i % 2].ap()
                    vector.tensor_tensor(
                        dst[:, 1:], prev[:, 1:], prev[:, :-1],
                        mybir.AluOpType.min).then_inc(dve_sem, 1)
                    continue
                # scan row i: data0 = m0 (i=0) or in-place S[s][i%2]
                m_ap = m0.ap() if i == 0 else S[s][i % 2].ap()
                need = act_through_square(u, i)
                if need > act_waited:
                    vector.wait_ge(act_sem, need)
                    act_waited = need
                vector.tensor_tensor_scan(
                    S[s][i % 2].ap(), m_ap, D[s][i % 2].ap(),
                    float(big),
                    op0=mybir.AluOpType.min,
                    op1=mybir.AluOpType.add).then_inc(dve_sem, 1)

        @block.scalar
        def _(scalar):
            scalar.wait_ge(dma0_sem, 32)  # negK + X0
            dve_waited = 0
            dma_waited = 0
            x1_waited = False
            for kind, u, i in act_ops:
                cc, wc = UNITS_R[u]
                s = u % SLOTS
                if cc == 1 and not x1_waited:
                    scalar.wait_ge(dma_sem, 16)  # X1
                    x1_waited = True
                if kind == "sq":
                    xt = X[cc].ap()
                    win = bass.AP(xt.tensor, xt.offset + 5 * NWC * wc,
                                  [list(xt.ap[0]), [5, NWC], [1, PROC]])
                    # WAR: D[s][i%2] was last read by an earlier scan
                    if i >= 2:
                        need = dve_through_scan(u, i - 2)
                    elif u >= SLOTS:
                        need = dve_through_scan(u - SLOTS, 8 + i)
                    else:
                        need = 0
                    if need > dve_waited:
                        scalar.wait_ge(dve_sem, need)
                        dve_waited = need
                    d_seg = D[s][i % 2].ap().rearrange(
                        "p (w s) -> p w s", s=SEG)
                    scalar.activation(
                        d_seg[:, :, 1:], win,
                        mybir.ActivationFunctionType.Square,
                        bias=negK.ap()[:, i:i + 1],
                        scale=1.0).then_inc(act_sem, 1)
                else:
                    # extract: cell j=19 lives at segment offset 20; final
                    # row (i=9, odd) lands in S[s][1]
                    need = dve_through_scan(u, K - 1)
                    if need > dve_waited:
                        scalar.wait_ge(dve_sem, need)
                        dve_waited = need
                    if u >= SLOTS:
                        dneed = dma_through_out(u - SLOTS)
                        if dneed > dma_waited:
                            scalar.wait_ge(dma_sem, dneed)
                            dma_waited = dneed
                    s_seg = S[s][1].ap().rearrange("p (w s) -> p w s",
                                                   s=SEG)
                    scalar.copy(OB[s].ap(), s_seg[:, :, SEG - 1]).then_inc(
                        act_sem, 1)
    return nc


_NC_CACHE = None


def kernel(x: np.ndarray, kernels: np.ndarray) -> np.ndarray:
    global _NC_CACHE
    if _NC_CACHE is None:
        _NC_CACHE = _build_nc()
    nc = _NC_CACHE
    x = np.ascontiguousarray(x, dtype=np.float32)
    negk = np.ascontiguousarray(-np.asarray(kernels, dtype=np.float32))
    in_maps = [{"x": x[b], "negk": negk} for b in range(B)]
    res = run_bass_kernel_spmd(nc, in_maps, core_ids=list(range(B)))
    out = np.stack([res.results[b]["out"] for b in range(B)], axis=0)
    return out[:, :, :NW]
